# revision 2
# baseline (speedup 1.0000x reference)
"""Trainium2 Bass kernel for nn_Bottleneck (sparse 3x3 gather-GEMM bottleneck block).

Strategy (8 NeuronCores, zero cross-core communication):
  - The neighbor graph (19% occupancy on a 1024^2 grid, Moore stencil) is far
    below the percolation threshold: connected components are tiny (max ~54).
    Host assigns whole components to cores, so every neighbor reference is
    core-local.
  - Per core, everything is computed channel-major ([C, tokens] tiles).
    LayerNorm centering is folded into the conv weights (W' = W @ (I - 1/C)),
    so conv outputs are pre-centered; variance comes from a ones-matmul over
    the squared activations; rstd is broadcast along partitions via a small
    replicating DMA; scale/shift fold into scalar_tensor_tensor + the Gelu
    activation's per-partition scale/bias.
  - conv2's 3x3 gather uses the GPSIMD dma_gather transpose mode over a
    point-major bf16 table in SBUF (built with DMA transposes), yielding
    channel-major gathered operands directly. The (0,0) tap needs no gather.
  - LN3 variance is computed *before* conv3 via a Cholesky factor L with
    L L^T = W3' W3'^T / 256, so var3 = ||L^T h2n||^2.
"""

import sys

sys.path.insert(0, "/opt/trn_rl_repo")

import numpy as np

import concourse.bass as bass
import concourse.tile as tile
from concourse import bacc as bacc_mod
from concourse import library_config, mybir
from concourse.bass_utils import run_bass_kernel_spmd

# Problem constants (hardcoded per contract).
N = 200000
C_IN = 256
C_MID = 64
EPS = 1e-6
NCORES = 8

# Per-core padded token count: 25 pairs of 1024 tokens.
T = 25600
PAIR = 1024
SUB = 512
NPAIR = T // PAIR          # 25
NGRP = (NPAIR + 1) // 2    # 13 groups of <=2 pairs
NBLK = T // 128            # 200 table blocks
SENT = T                   # sentinel token id -> zeroed table rank
NRANK = NBLK + 1           # 201 ranks in the gather table
KS = list(range(9))        # all 9 taps via gather
GIDX = 9 * PAIR            # gather indices per pair

f32 = mybir.dt.float32
f32r = mybir.dt.float32r
bf16 = mybir.dt.bfloat16
i16 = mybir.dt.int16
AF = mybir.ActivationFunctionType
OP = mybir.AluOpType

_NC_CACHE = {}


def _bcast_ap(src: bass.AP, ap_dims):
    """Manual access pattern with explicit [step, count] dims over src."""
    return bass.AP(tensor=src.tensor, offset=src.offset, ap=ap_dims)


def build_nc():
    if "nc" in _NC_CACHE:
        return _NC_CACHE["nc"]
    nc = bacc_mod.Bacc(None, target_bir_lowering=False, debug=False)

    x_t = nc.declare_dram_parameter("x_t", [2, 128, T], f32, isOutput=False)
    w1 = nc.declare_dram_parameter("w1", [128, 2, C_MID], bf16, isOutput=False)
    w2 = nc.declare_dram_parameter("w2", [C_MID, 9, C_MID], bf16, isOutput=False)
    w3 = nc.declare_dram_parameter("w3", [128, C_IN], bf16, isOutput=False)
    lmat = nc.declare_dram_parameter("lmat", [128, C_MID], bf16, isOutput=False)
    es = nc.declare_dram_parameter("es", [128, 2, 4], f32r, isOutput=False)
    g1r = nc.declare_dram_parameter("g1r", [128, 1], f32, isOutput=False)
    b1r = nc.declare_dram_parameter("b1r", [128, 1], f32, isOutput=False)
    g2r = nc.declare_dram_parameter("g2r", [128, 1], f32, isOutput=False)
    b2r = nc.declare_dram_parameter("b2r", [128, 1], f32, isOutput=False)
    g3r = nc.declare_dram_parameter("g3r", [128, 2], f32, isOutput=False)
    b3r = nc.declare_dram_parameter("b3r", [128, 2], f32, isOutput=False)
    idx = nc.declare_dram_parameter("idx", [NPAIR, 2, 128, GIDX // 32], i16, isOutput=False)
    y_t = nc.declare_dram_parameter("y_t", [2, 128, T], f32, isOutput=True)

    from contextlib import ExitStack

    with ExitStack() as ctx:
        tc = ctx.enter_context(tile.TileContext(nc))
        consts = ctx.enter_context(tc.tile_pool(name="consts", bufs=1))
        tablep = ctx.enter_context(tc.tile_pool(name="table", bufs=1))
        h1p = ctx.enter_context(tc.tile_pool(name="h1cm", bufs=1))
        xp = ctx.enter_context(tc.tile_pool(name="xin", bufs=2))
        sqp = ctx.enter_context(tc.tile_pool(name="sq", bufs=2))
        rsp = ctx.enter_context(tc.tile_pool(name="rstd", bufs=2))
        rbp = ctx.enter_context(tc.tile_pool(name="rbc", bufs=2))
        rb3p = ctx.enter_context(tc.tile_pool(name="rbc3", bufs=1))
        gp = ctx.enter_context(tc.tile_pool(name="gath", bufs=2))
        h2p = ctx.enter_context(tc.tile_pool(name="h2n", bufs=2))
        ip = ctx.enter_context(tc.tile_pool(name="idxp", bufs=1))
        yp = ctx.enter_context(tc.tile_pool(name="yout", bufs=2))
        psA = ctx.enter_context(tc.tile_pool(name="psA", bufs=2, space="PSUM"))
        psH = ctx.enter_context(tc.tile_pool(name="psH", bufs=2, space="PSUM"))
        psU = ctx.enter_context(tc.tile_pool(name="psU", bufs=1, space="PSUM"))
        psC = ctx.enter_context(tc.tile_pool(name="psC", bufs=2, space="PSUM"))
        psG = ctx.enter_context(tc.tile_pool(name="psG", bufs=1, space="PSUM"))
        drp = ctx.enter_context(tc.tile_pool(name="drscratch", bufs=2, space="DRAM"))
        if True:
            # ---- constants into SBUF ----
            w1_sb = consts.tile([128, 2, C_MID], bf16)
            nc.sync.dma_start(out=w1_sb[:], in_=w1[:])
            w2_sb = consts.tile([C_MID, 9, C_MID], bf16)
            nc.sync.dma_start(out=w2_sb[:], in_=w2[:])
            w3_sb = consts.tile([128, C_IN], bf16)
            nc.sync.dma_start(out=w3_sb[:], in_=w3[:])
            l_sb = consts.tile([128, C_MID], bf16)
            nc.sync.dma_start(out=l_sb[:], in_=lmat[:])
            es_sb = consts.tile([128, 2, 4], f32r)
            nc.sync.dma_start(out=es_sb[:], in_=es[:])
            esb_sb = consts.tile([128, 2, 4], bf16)
            nc.gpsimd.dma_start(out=esb_sb[:], in_=es[:].bitcast(f32))
            sc = {}
            for name, src in (("g1", g1r), ("b1", b1r), ("g2", g2r), ("b2", b2r)):
                t_ = consts.tile([128, 1], f32, tag=f"sc_{name}")
                nc.sync.dma_start(out=t_[:], in_=src[:])
                sc[name] = t_
            g3_sb = consts.tile([128, 2], f32)
            nc.sync.dma_start(out=g3_sb[:], in_=g3r[:])
            b3_sb = consts.tile([128, 2], f32)
            nc.sync.dma_start(out=b3_sb[:], in_=b3r[:])
            eps_sb = consts.tile([128, 1], f32)
            nc.vector.memset(eps_sb[:], EPS)

            # ---- persistent arrays ----
            idx_all = ip.tile([128, NPAIR, 2, GIDX // 32], i16)
            nc.sync.dma_start(
                out=idx_all[:], in_=idx[:].rearrange("n s p f -> p n s f")
            )
            table = tablep.tile([128, NRANK * 128], bf16)   # point-major gather table
            nc.vector.memset(table[:], 0.0)
            h1cm = h1p.tile([128, (T // PAIR) * SUB], bf16)  # pair-packed channel-major h1n

            def pairs_of(g):
                return [p for p in (2 * g, 2 * g + 1) if p < NPAIR]

            # =================== PHASE A: conv1 + LN1 + GELU ===================
            for g in range(NGRP):
                ps = pairs_of(g)
                gridA = psG.tile([128, SUB], f32, tag="grid")
                if len(ps) == 1:
                    nc.vector.memset(gridA[0:4, :], 1.0)
                a1s = {}
                for j, p in enumerate(ps):
                    x_sb = xp.tile([128, 2, PAIR], bf16, tag="xin")
                    nc.gpsimd.dma_start(
                        out=x_sb[:],
                        in_=x_t[:, :, p * PAIR:(p + 1) * PAIR].rearrange(
                            "c p f -> p c f"
                        ),
                    )
                    a1 = psA.tile([128, SUB], f32, tag="A1")
                    a1s[j] = a1
                    for s in range(2):
                        for c in range(2):
                            nc.tensor.matmul(
                                out=a1[64 * s:64 * s + 64, :],
                                lhsT=w1_sb[:, c, :],
                                rhs=x_sb[:, c, SUB * s:SUB * (s + 1)],
                                start=(c == 0),
                                stop=(c == 1),
                                tile_position=(0, 64 * s),
                            )
                    sq = sqp.tile([128, SUB], f32r, tag="sq")
                    nc.scalar.activation(out=sq[:], in_=a1[:], func=AF.Square)
                    nc.tensor.matmul(
                        out=gridA[0:4, :],
                        lhsT=es_sb[:, j, :],
                        rhs=sq[:],
                        start=(j == 0),
                        stop=(j == len(ps) - 1),
                    )
                rstd = rsp.tile([4, SUB], f32, tag="rstd")
                nc.scalar.activation(
                    out=rstd[:], in_=gridA[0:4, :], func=AF.Sqrt,
                    bias=eps_sb[0:4, :], scale=1.0 / C_MID,
                )
                nc.vector.reciprocal(out=rstd[:], in_=rstd[:])
                rsc = drp.tile([4, SUB], f32, tag="rsc")
                nc.sync.dma_start(out=rsc[:], in_=rstd[:])
                rb = rbp.tile([128, 2, SUB], f32, tag="rb1")
                for j in range(len(ps)):
                    for s in range(2):
                        nc.sync.dma_start(
                            out=rb[64 * s:64 * s + 64, j, :],
                            in_=_bcast_ap(
                                rsc[2 * j + s:2 * j + s + 1, :],
                                [[0, 64], [1, SUB]],
                            ),
                        )
                for j, p in enumerate(ps):
                    h1s = h1cm[:, p * SUB:(p + 1) * SUB]
                    nc.vector.scalar_tensor_tensor(
                        out=h1s, in0=a1s[j][:], scalar=sc["g1"][:],
                        in1=rb[:, j, :], op0=OP.mult, op1=OP.mult,
                    )
                    nc.scalar.activation(
                        out=h1s, in_=h1s, func=AF.Gelu, bias=sc["b1"][:],
                    )
                    # point-major bf16 table via DMA transposes (8 blocks/pair)
                    for bl in range(8):
                        b = 8 * p + bl
                        s = bl // 4
                        c0 = p * SUB + 128 * (bl % 4)
                        nc.sync.dma_start_transpose(
                            out=table[:, 128 * b:128 * b + 64],
                            in_=h1cm[64 * s:64 * s + 64, c0:c0 + 128],
                        )

            # =================== PHASE B: conv2 + LN2 + conv3 + LN3 ============
            nidx_reg = nc.gpsimd.to_reg(GIDX // 2)
            for g in range(NGRP):
                ps = pairs_of(g)
                gridB = psG.tile([128, SUB], f32, tag="grid")
                if len(ps) == 1:
                    nc.vector.memset(gridB[0:4, :], 1.0)
                    nc.vector.memset(gridB[64:68, :], 1.0)
                h2ps = {}
                for j, p in enumerate(ps):
                    h2 = psH.tile([128, SUB], f32, tag="H2")
                    h2ps[j] = h2
                    for s in range(2):
                        gath = gp.tile([128, 1, GIDX // 2], bf16, tag="gath")
                        nc.gpsimd.dma_gather(
                            out_ap=gath[:],
                            in_ap=table[:],
                            idxs_ap=idx_all[:, p, s, :],
                            num_idxs=GIDX // 2,
                            num_idxs_reg=nidx_reg,
                            elem_size=128,
                            transpose=True,
                            sbuf_tokens_per_rank=128,
                            sbuf_free_dim_per_rank=256,
                            sbuf_free_dim_pad_per_rank=0,
                            sbuf_byte_offset=0,
                            single_packet=False,
                        )
                        for ki, k in enumerate(KS):
                            nc.tensor.matmul(
                                out=h2[64 * s:64 * s + 64, :],
                                lhsT=w2_sb[:, k, :],
                                rhs=gath[0:64, 0, ki * SUB:(ki + 1) * SUB],
                                start=(ki == 0),
                                stop=(ki == 8),
                                tile_position=(0, 64 * s),
                            )
                    sq = sqp.tile([128, SUB], f32r, tag="sq")
                    nc.scalar.activation(out=sq[:], in_=h2[:], func=AF.Square)
                    nc.tensor.matmul(
                        out=gridB[0:4, :],
                        lhsT=es_sb[:, j, :],
                        rhs=sq[:],
                        start=(j == 0),
                        stop=(j == len(ps) - 1),
                    )
                rstd2 = rsp.tile([4, SUB], f32, tag="rstd")
                nc.scalar.activation(
                    out=rstd2[:], in_=gridB[0:4, :], func=AF.Sqrt,
                    bias=eps_sb[0:4, :], scale=1.0 / C_MID,
                )
                nc.vector.reciprocal(out=rstd2[:], in_=rstd2[:])
                rsc2 = drp.tile([4, SUB], f32, tag="rsc")
                nc.sync.dma_start(out=rsc2[:], in_=rstd2[:])
                rb2 = rbp.tile([128, 2, SUB], f32, tag="rb1")
                for j in range(len(ps)):
                    for s in range(2):
                        nc.sync.dma_start(
                            out=rb2[64 * s:64 * s + 64, j, :],
                            in_=_bcast_ap(
                                rsc2[2 * j + s:2 * j + s + 1, :],
                                [[0, 64], [1, SUB]],
                            ),
                        )
                h2ns = {}
                for j, p in enumerate(ps):
                    h2n = h2p.tile([128, SUB], bf16, tag="h2n")
                    h2ns[j] = h2n
                    nc.vector.scalar_tensor_tensor(
                        out=h2n[:], in0=h2ps[j][:], scalar=sc["g2"][:],
                        in1=rb2[:, j, :], op0=OP.mult, op1=OP.mult,
                    )
                    nc.scalar.activation(
                        out=h2n[:], in_=h2n[:], func=AF.Gelu, bias=sc["b2"][:],
                    )
                    u = psU.tile([128, SUB], f32, tag="U")
                    for s in range(2):
                        nc.tensor.matmul(
                            out=u[64 * s:64 * s + 64, :],
                            lhsT=l_sb[64 * s:64 * s + 64, :],
                            rhs=h2n[64 * s:64 * s + 64, :],
                            start=True,
                            stop=True,
                            tile_position=(64 * s, 64 * s),
                        )
                    squ = sqp.tile([128, SUB], bf16, tag="squ")
                    nc.scalar.activation(out=squ[:], in_=u[:], func=AF.Square)
                    nc.tensor.matmul(
                        out=gridB[64:68, :],
                        lhsT=esb_sb[:, j, :],
                        rhs=squ[:],
                        start=(j == 0),
                        stop=(j == len(ps) - 1),
                        tile_position=(0, 64),
                    )
                rstd3 = rsp.tile([4, SUB], f32, tag="rstd3")
                nc.scalar.activation(
                    out=rstd3[:], in_=gridB[64:68, :], func=AF.Sqrt,
                    bias=eps_sb[0:4, :], scale=1.0,
                )
                nc.vector.reciprocal(out=rstd3[:], in_=rstd3[:])
                rsc3 = drp.tile([4, SUB], f32, tag="rsc")
                nc.sync.dma_start(out=rsc3[:], in_=rstd3[:])
                rb3 = rb3p.tile([128, 2, 2, SUB], f32, tag="rb3")
                for j in range(len(ps)):
                    for s in range(2):
                        nc.sync.dma_start(
                            out=rb3[:, j, s, :],
                            in_=_bcast_ap(
                                rsc3[2 * j + s:2 * j + s + 1, :],
                                [[0, 128], [1, SUB]],
                            ),
                        )
                for j, p in enumerate(ps):
                    y_sb = yp.tile([128, 2, PAIR], f32, tag="y")
                    for s in range(2):
                        for c in range(2):
                            a3 = psC.tile([128, SUB], f32, tag="A3")
                            nc.tensor.matmul(
                                out=a3[:],
                                lhsT=w3_sb[64 * s:64 * s + 64,
                                           128 * c:128 * (c + 1)],
                                rhs=h2ns[j][64 * s:64 * s + 64, :],
                                start=True,
                                stop=True,
                                tile_position=(64 * s, 0),
                            )
                            nc.vector.scalar_tensor_tensor(
                                out=y_sb[:, c, SUB * s:SUB * (s + 1)],
                                in0=a3[:], scalar=g3_sb[:, c:c + 1],
                                in1=rb3[:, j, s, :], op0=OP.mult, op1=OP.mult,
                            )
                    nc.gpsimd.dma_start(
                        out=y_sb[:],
                        in_=x_t[:, :, p * PAIR:(p + 1) * PAIR].rearrange(
                            "c p f -> p c f"
                        ),
                        accum_op=OP.add,
                    )
                    for c in range(2):
                        nc.scalar.activation(
                            out=y_sb[:, c, :], in_=y_sb[:, c, :],
                            func=AF.Gelu, bias=b3_sb[:, c:c + 1],
                        )
                    nc.sync.dma_start(
                        out=y_t[:, :, p * PAIR:(p + 1) * PAIR].rearrange(
                            "c p f -> p c f"
                        ),
                        in_=y_sb[:],
                    )

    nc.compile()
    _NC_CACHE["nc"] = nc
    return nc


# ======================= host-side sharding =======================

def _components(nbr):
    """Connected-component labels via vectorized min-label propagation."""
    lab = np.arange(N, dtype=np.int64)
    ks = [k for k in range(9) if k != 4]
    valid = [(nbr[k] < N) for k in ks]
    nbrs = [nbr[k].astype(np.int64) for k in ks]
    for _ in range(200):
        new = lab.copy()
        for k in range(len(ks)):
            v = valid[k]
            cand = lab[nbrs[k][v]]
            np.minimum.at(new, np.nonzero(v)[0], cand)
        # also pull own label forward through one hop of indirection
        new = np.minimum(new, new[new])
        if np.array_equal(new, lab):
            break
        lab = new
    # canonicalize
    while True:
        new = lab[lab]
        if np.array_equal(new, lab):
            break
        lab = new
    return lab


def _shard(nbr):
    lab = _components(nbr)
    comp_ids, comp_inv, comp_sizes = np.unique(
        lab, return_inverse=True, return_counts=True
    )
    order = np.argsort(comp_sizes)[::-1]
    import heapq

    heap = [(0, c) for c in range(NCORES)]
    heapq.heapify(heap)
    comp_core = np.empty(len(comp_ids), dtype=np.int64)
    for ci in order:
        load, core = heapq.heappop(heap)
        comp_core[ci] = core
        heapq.heappush(heap, (load + int(comp_sizes[ci]), core))
    point_core = comp_core[comp_inv]
    ids_per_core = [np.nonzero(point_core == c)[0] for c in range(NCORES)]
    for c in range(NCORES):
        assert len(ids_per_core[c]) <= T, f"core {c} overloaded: {len(ids_per_core[c])}"
    return ids_per_core


def _prep_core(x, nbr, ids):
    n = len(ids)
    glob2loc = np.full(N + 1, SENT, dtype=np.int64)
    glob2loc[ids] = np.arange(n)
    xl = np.zeros((T, C_IN), dtype=np.float32)
    xl[:n] = x[ids]
    x_t = np.ascontiguousarray(xl.T.reshape(2, 128, T))
    nbl = np.full((9, T), SENT, dtype=np.int64)
    nbl[:, :n] = glob2loc[np.where(nbr[:, ids] < N, nbr[:, ids], N)]
    # idx blob: per (pair, subtile): k-major/token order, 16-wrapped, x8 replicated
    flat = np.empty((NPAIR, 2, GIDX // 2), dtype=np.int16)
    for ki, k in enumerate(KS):
        seg = nbl[k].reshape(NPAIR, 2, SUB).astype(np.int16)
        flat[:, :, ki * SUB:(ki + 1) * SUB] = seg
    wrapped = flat.reshape(NPAIR, 2, GIDX // 32, 16).transpose(0, 1, 3, 2)
    idx_blob = np.ascontiguousarray(np.tile(wrapped, (1, 1, 8, 1)))  # [NPAIR,2,128,288]
    return x_t, idx_blob, n


def _prep_weights(W1, W2, W3, g1, b1, g2, b2, g3, b3):
    W1 = np.asarray(W1, np.float64)
    W2 = np.asarray(W2, np.float64)
    W3 = np.asarray(W3, np.float64)

    def center(w, C):
        return w - w.mean(axis=-1, keepdims=True)

    W1c = center(W1, C_MID)          # [256, 64]
    W2c = center(W2, C_MID)          # [9, 64, 64]
    W3c = center(W3, C_IN)           # [64, 256]
    import ml_dtypes as _mld
    w1 = np.ascontiguousarray(
        W1c.reshape(2, 128, C_MID).transpose(1, 0, 2).astype(np.float32)
        .astype(_mld.bfloat16)
    )
    import ml_dtypes
    w2 = np.ascontiguousarray(
        W2c.transpose(1, 0, 2).astype(np.float32).astype(ml_dtypes.bfloat16)
    )  # [64, 9, 64]
    w3 = np.ascontiguousarray(
        np.tile(W3c.astype(np.float32), (2, 1)).astype(_mld.bfloat16)
    )  # [128, 256] rows 64-127 replicated
    M3 = (W3c @ W3c.T) / C_IN
    L = np.linalg.cholesky(M3 + 1e-12 * np.eye(C_MID))
    lmat = np.ascontiguousarray(
        np.tile(L.astype(np.float32), (2, 1)).astype(_mld.bfloat16)
    )  # [128, 64]

    def rep2(v):
        return np.ascontiguousarray(
            np.tile(np.asarray(v, np.float32).reshape(C_MID), 2).reshape(128, 1)
        )

    g1r, b1r, g2r, b2r = rep2(g1), rep2(b1), rep2(g2), rep2(b2)
    g3r = np.ascontiguousarray(np.asarray(g3, np.float32).reshape(2, 128).T)
    b3r = np.ascontiguousarray(np.asarray(b3, np.float32).reshape(2, 128).T)
    es = np.zeros((128, 2, 4), np.float32)
    for j in range(2):
        for p in range(128):
            es[p, j, 2 * j + p // 64] = 1.0
    return w1, w2, w3, lmat, g1r, b1r, g2r, b2r, g3r, b3r, es


def prep_in_maps(inputs):
    x = np.asarray(inputs["x"], np.float32)
    nbr = np.asarray(inputs["neighbor_idx"])
    (w1, w2, w3, lmat, g1r, b1r, g2r, b2r, g3r, b3r, es) = _prep_weights(
        inputs["W1"], inputs["W2"], inputs["W3"], inputs["g1"], inputs["b1"],
        inputs["g2"], inputs["b2"], inputs["g3"], inputs["b3"],
    )
    ids_per_core = _shard(nbr)
    in_maps = []
    metas = []
    for c in range(NCORES):
        x_t, idx_blob, n = _prep_core(x, nbr, ids_per_core[c])
        metas.append((ids_per_core[c], n))
        in_maps.append(
            dict(
                x_t=x_t, idx=idx_blob, w1=w1, w2=w2, w3=w3, lmat=lmat,
                es=es, g1r=g1r, b1r=b1r, g2r=g2r, b2r=b2r, g3r=g3r, b3r=b3r,
            )
        )
    return in_maps, metas


def kernel(x, W1, W2, W3, g1, b1, g2, b2, g3, b3, neighbor_idx):
    in_maps, metas = prep_in_maps(
        dict(
            x=x, W1=W1, W2=W2, W3=W3, g1=g1, b1=b1, g2=g2, b2=b2,
            g3=g3, b3=b3, neighbor_idx=neighbor_idx,
        )
    )
    nc = build_nc()
    res = run_bass_kernel_spmd(nc, in_maps, core_ids=list(range(NCORES)))
    y = np.empty((N, C_IN), dtype=np.float32)
    for c in range(NCORES):
        yt = res.results[c]["y_t"]  # [2, 128, T]
        ids, n = metas[c]
        yl = yt.reshape(C_IN, T).T  # [T, 256]
        y[ids] = yl[:n]
    return y



# revision 11
# speedup vs baseline: 2.1137x; 2.1137x over previous
"""Trainium2 Bass kernel for nn_Bottleneck (sparse 3x3 gather-GEMM bottleneck block).

Strategy (8 NeuronCores, zero cross-core communication):
  - Connected components of the 19%-occupancy Moore-stencil graph are tiny;
    host assigns whole components to cores so every neighbor is core-local.
  - Channel-major compute; LN centering folded into conv weights; variance
    via ones-matmul; rstd broadcast via small DRAM-bounce DMAs; LN3 variance
    before conv3 via a Cholesky factor of W3c W3c^T / 256.
  - conv2 is EDGE-COMPACTED: instead of gathering all 9 taps x T tokens
    (230k descriptor generations on the Q7 SWDGE — the old bottleneck), we
    gather only real edges per tap (~4.9k/tap), multiply with the tap weight
    in a flipped matmul (lhsT = gathered activations) yielding token-major
    products, and dma_scatter_add them into SBUF parity accumulators.
    The center tap needs no gather: it is a dense matmul over h1.
  - x and y are bf16 in DRAM (host converts) to halve HBM traffic.
"""

import sys

sys.path.insert(0, "/opt/trn_rl_repo")

import numpy as np

import concourse.bass as bass
import concourse.tile as tile
from concourse import bacc as bacc_mod
from concourse import library_config, mybir
from concourse.bass_utils import run_bass_kernel_spmd

# Problem constants (hardcoded per contract).
N = 200000
C_IN = 256
C_MID = 64
EPS = 1e-6
NCORES = 8

# Per-core padded token count: 25 pairs of 1024 tokens.
T = 25600
PAIR = 1024
SUB = 512
NPAIR = T // PAIR          # 25
NGRP = (NPAIR + 1) // 2    # 13 groups of <=2 pairs
NBLK = T // 128            # 200 table blocks
SENT = T                   # sentinel token id -> zeroed table rank
NRANK = NBLK + 1           # 201 ranks in the gather table
KS8 = [0, 1, 2, 3, 5, 6, 7, 8]  # non-center taps
NK = 4992                  # padded edges per tap (max measured 4852)
NCH = NK // 128            # 39 matmul chunks per tap
NG = NBLK // 2 + 1         # accumulator groups per parity (incl. trash)
TRASH = T                  # scatter pad target -> group 100 (trash)

f32 = mybir.dt.float32
f32r = mybir.dt.float32r
bf16 = mybir.dt.bfloat16
i16 = mybir.dt.int16
AF = mybir.ActivationFunctionType
OP = mybir.AluOpType

_NC_CACHE = {}


def _bcast_ap(src: bass.AP, ap_dims):
    """Manual access pattern with explicit [step, count] dims over src."""
    return bass.AP(tensor=src.tensor, offset=src.offset, ap=ap_dims)


def build_nc():
    if "nc" in _NC_CACHE:
        return _NC_CACHE["nc"]
    nc = bacc_mod.Bacc(None, target_bir_lowering=False, debug=False)

    x_t = nc.declare_dram_parameter("x_t", [2, 128, T], bf16, isOutput=False)
    w1 = nc.declare_dram_parameter("w1", [128, 2, C_MID], bf16, isOutput=False)
    w2 = nc.declare_dram_parameter("w2", [C_MID, 9, C_MID], bf16, isOutput=False)
    w2c = nc.declare_dram_parameter("w2c", [128, C_MID], bf16, isOutput=False)
    w3 = nc.declare_dram_parameter("w3", [128, C_IN], bf16, isOutput=False)
    lmat = nc.declare_dram_parameter("lmat", [128, C_MID], bf16, isOutput=False)
    es = nc.declare_dram_parameter("es", [128, 2, 4], f32r, isOutput=False)
    g1r = nc.declare_dram_parameter("g1r", [128, 1], f32, isOutput=False)
    b1r = nc.declare_dram_parameter("b1r", [128, 1], f32, isOutput=False)
    g2r = nc.declare_dram_parameter("g2r", [128, 1], f32, isOutput=False)
    b2r = nc.declare_dram_parameter("b2r", [128, 1], f32, isOutput=False)
    g3r = nc.declare_dram_parameter("g3r", [128, 2], f32, isOutput=False)
    b3r = nc.declare_dram_parameter("b3r", [128, 2], f32, isOutput=False)
    identf = nc.declare_dram_parameter("identf", [128, 128], f32, isOutput=False)
    identb = nc.declare_dram_parameter("identb", [128, 128], bf16, isOutput=False)
    gidx = nc.declare_dram_parameter("gidx", [8, 128, NK // 16], i16, isOutput=False)
    sidx = nc.declare_dram_parameter("sidx", [8, 128, NK // 16], i16, isOutput=False)
    y_t = nc.declare_dram_parameter("y_t", [2, 128, T], bf16, isOutput=True)
    import os
    dbg = os.environ.get("KDBG", "0") == "1"
    if dbg:
        acc_out = nc.declare_dram_parameter(
            "acc_out", [128, 2, NG, C_MID], f32, isOutput=True
        )
        h2f_out = nc.declare_dram_parameter(
            "h2f_out", [128, NPAIR, SUB], f32, isOutput=True
        )
        h1p_out = nc.declare_dram_parameter(
            "h1p_out", [128, NPAIR, SUB], bf16, isOutput=True
        )
        pst_out = nc.declare_dram_parameter(
            "pst_out", [64, 2, SUB], f32, isOutput=True
        )
        tab_out = nc.declare_dram_parameter(
            "tab_out", [128, NRANK * 128], bf16, isOutput=True
        )

    from contextlib import ExitStack

    with ExitStack() as ctx:
        tc = ctx.enter_context(tile.TileContext(nc))
        consts = ctx.enter_context(tc.tile_pool(name="consts", bufs=1))
        tablep = ctx.enter_context(tc.tile_pool(name="table", bufs=1))
        accp = ctx.enter_context(tc.tile_pool(name="acc", bufs=1))
        xp = ctx.enter_context(tc.tile_pool(name="xin", bufs=2))
        sqp = ctx.enter_context(tc.tile_pool(name="sq", bufs=2))
        rsp = ctx.enter_context(tc.tile_pool(name="rstd", bufs=2))
        rbp = ctx.enter_context(tc.tile_pool(name="rbc", bufs=2))
        drp = ctx.enter_context(tc.tile_pool(name="drscratch", bufs=2, space="DRAM"))

        # ---- constants into SBUF ----
        w1_sb = consts.tile([128, 2, C_MID], bf16)
        nc.sync.dma_start(out=w1_sb[:], in_=w1[:])
        w2_sb = consts.tile([C_MID, 9, C_MID], bf16)
        nc.sync.dma_start(out=w2_sb[:], in_=w2[:])
        w2c_sb = consts.tile([128, C_MID], bf16)
        nc.sync.dma_start(out=w2c_sb[:], in_=w2c[:])
        w3_sb = consts.tile([128, C_IN], bf16)
        nc.sync.dma_start(out=w3_sb[:], in_=w3[:])
        l_sb = consts.tile([128, C_MID], bf16)
        nc.sync.dma_start(out=l_sb[:], in_=lmat[:])
        es_sb = consts.tile([128, 2, 4], f32r)
        nc.sync.dma_start(out=es_sb[:], in_=es[:])
        esb_sb = consts.tile([128, 2, 4], bf16)
        nc.gpsimd.dma_start(out=esb_sb[:], in_=es[:].bitcast(f32))
        identf_sb = consts.tile([128, 128], f32)
        nc.sync.dma_start(out=identf_sb[:], in_=identf[:])
        identb_sb = consts.tile([128, 128], bf16)
        nc.sync.dma_start(out=identb_sb[:], in_=identb[:])
        sc = {}
        for name, src in (("g1", g1r), ("b1", b1r), ("g2", g2r), ("b2", b2r)):
            t_ = consts.tile([128, 1], f32, tag=f"sc_{name}")
            nc.sync.dma_start(out=t_[:], in_=src[:])
            sc[name] = t_
        g3_sb = consts.tile([128, 2], f32)
        nc.sync.dma_start(out=g3_sb[:], in_=g3r[:])
        b3_sb = consts.tile([128, 2], f32)
        nc.sync.dma_start(out=b3_sb[:], in_=b3r[:])
        eps_sb = consts.tile([128, 1], f32)
        nc.vector.memset(eps_sb[:], EPS)
        ones_sb = consts.tile([128, 1], f32)
        nc.vector.memset(ones_sb[:], 1.0)

        table = tablep.tile([128, NRANK * 128], bf16)   # point-major gather table
        nc.vector.memset(table[:], 0.0)

        def pairs_of(g):
            return [p for p in (2 * g, 2 * g + 1) if p < NPAIR]

        # =================== PHASE A: conv1 + LN1 + GELU -> table ============
        with tc.tile_pool(name="h1a", bufs=4) as h1ap, \
             tc.tile_pool(name="psA", bufs=2, space="PSUM") as psA, \
             tc.tile_pool(name="psGA", bufs=2, space="PSUM") as psGA:
            for g in range(NGRP):
                ps = pairs_of(g)
                gridA = psGA.tile([128, SUB], f32, tag="grid")
                if len(ps) == 1:
                    nc.vector.memset(gridA[0:4, :], 1.0)
                a1s = {}
                for j, p in enumerate(ps):
                    x_sb = xp.tile([128, 2, PAIR], bf16, tag="xin")
                    nc.sync.dma_start(
                        out=x_sb[:],
                        in_=x_t[:, :, p * PAIR:(p + 1) * PAIR].rearrange(
                            "c p f -> p c f"
                        ),
                    )
                    a1 = psA.tile([128, SUB], f32, tag="A1")
                    a1s[j] = a1
                    for s in range(2):
                        for c in range(2):
                            nc.tensor.matmul(
                                out=a1[64 * s:64 * s + 64, :],
                                lhsT=w1_sb[:, c, :],
                                rhs=x_sb[:, c, SUB * s:SUB * (s + 1)],
                                start=(c == 0),
                                stop=(c == 1),
                                tile_position=(0, 64 * s),
                            )
                    sq = sqp.tile([128, SUB], f32r, tag="sq")
                    nc.scalar.activation(out=sq[:], in_=a1[:], func=AF.Square)
                    nc.tensor.matmul(
                        out=gridA[0:4, :],
                        lhsT=es_sb[:, j, :],
                        rhs=sq[:],
                        start=(j == 0),
                        stop=(j == len(ps) - 1),
                    )
                rstd = rsp.tile([4, SUB], f32, tag="rstd")
                nc.scalar.activation(
                    out=rstd[:], in_=gridA[0:4, :], func=AF.Sqrt,
                    bias=eps_sb[0:4, :], scale=1.0 / C_MID,
                )
                nc.vector.reciprocal(out=rstd[:], in_=rstd[:])
                rsc = drp.tile([4, SUB], f32, tag="rsc")
                nc.sync.dma_start(out=rsc[:], in_=rstd[:])
                rb = rbp.tile([128, 2, SUB], f32, tag="rb1")
                for j in range(len(ps)):
                    for s in range(2):
                        nc.scalar.dma_start(
                            out=rb[64 * s:64 * s + 64, j, :],
                            in_=_bcast_ap(
                                rsc[2 * j + s:2 * j + s + 1, :],
                                [[0, 64], [1, SUB]],
                            ),
                        )
                for j, p in enumerate(ps):
                    h1a = h1ap.tile([128, SUB], bf16, tag="h1a")
                    nc.vector.scalar_tensor_tensor(
                        out=h1a[:], in0=a1s[j][:], scalar=sc["g1"][:],
                        in1=rb[:, j, :], op0=OP.mult, op1=OP.mult,
                    )
                    nc.scalar.activation(
                        out=h1a[:], in_=h1a[:], func=AF.Gelu, bias=sc["b1"][:],
                    )
                    # point-major bf16 table via DMA transposes (8 blocks/pair)
                    for bl in range(8):
                        b = 8 * p + bl
                        s = bl // 4
                        c0 = 128 * (bl % 4)
                        nc.sync.dma_start_transpose(
                            out=table[:, 128 * b:128 * b + 64],
                            in_=h1a[64 * s:64 * s + 64, c0:c0 + 128],
                        )

        # =================== PHASE B: edge-compacted conv2 taps ==============
        acc = accp.tile([128, 2, NG, C_MID], f32)
        nc.vector.memset(acc[:], 0.0)
        nidx = nc.gpsimd.to_reg(NK)
        zreg = nc.gpsimd.to_reg(0)
        with tc.tile_pool(name="gidxp", bufs=2) as gip, \
             tc.tile_pool(name="sidxp", bufs=2) as sip, \
             tc.tile_pool(name="gath", bufs=2) as gp, \
             tc.tile_pool(name="ssrc", bufs=1) as sp, \
             tc.tile_pool(name="psB", bufs=2, space="PSUM") as psB:
            for k8, k in enumerate(KS8):
                gi = gip.tile([128, NK // 16], i16, tag="gi")
                nc.sync.dma_start(out=gi[:], in_=gidx[k8])
                si = sip.tile([128, NK // 16], i16, tag="si")
                nc.sync.dma_start(out=si[:], in_=sidx[k8])
                gath = gp.tile([128, 1, NK], bf16, tag="gath")
                nc.gpsimd.dma_gather(
                    out_ap=gath[:],
                    in_ap=table[:],
                    idxs_ap=gi[:],
                    num_idxs=NK,
                    num_idxs_reg=nidx,
                    elem_size=128,
                    transpose=True,
                    sbuf_tokens_per_rank=128,
                    sbuf_free_dim_per_rank=256,
                    sbuf_free_dim_pad_per_rank=0,
                    sbuf_byte_offset=0,
                    single_packet=False,
                )
                src = sp.tile([128, NCH, C_MID], f32, tag="src")
                for gch in range(0, NCH, 8):
                    gg = min(8, NCH - gch)
                    psb = psB.tile([128, SUB], f32, tag="psb")
                    for j in range(gg):
                        ch = gch + j
                        nc.tensor.matmul(
                            out=psb[:, 64 * j:64 * (j + 1)],
                            lhsT=gath[0:64, 0, 128 * ch:128 * (ch + 1)],
                            rhs=w2_sb[:, k, :],
                            start=True,
                            stop=True,
                        )
                    nc.scalar.copy(
                        out=src[:, gch:gch + gg, :].rearrange("p a b -> p (a b)"),
                        in_=psb[:, 0:64 * gg],
                    )
                nc.gpsimd.dma_scatter_add(
                    acc[:, 0, :, :],
                    src[:],
                    si[:],
                    NK,
                    nidx,
                    C_MID,
                    parity_reg=zreg,
                    out_ap_other=acc[:, 1, :, :],
                    sbuf_tokens_per_rank=128,
                    single_packet=False,
                )

        if dbg:
            nc.sync.dma_start(out=acc_out[:], in_=acc[:])
            nc.sync.dma_start(out=tab_out[:], in_=table[:])

        # ====== PHASE C: center tap + LN2 + GELU + conv3 + LN3 + res ========
        with tc.tile_pool(name="h1pp", bufs=2) as h1pp, \
             tc.tile_pool(name="h2fp", bufs=2) as h2fp, \
             tc.tile_pool(name="h2np", bufs=2) as h2p, \
             tc.tile_pool(name="rb3p", bufs=1) as rb3p, \
             tc.tile_pool(name="ysb", bufs=2) as yp, \
             tc.tile_pool(name="yout", bufs=2) as yop, \
             tc.tile_pool(name="psH1", bufs=1, space="PSUM") as psH1p, \
             tc.tile_pool(name="psT", bufs=1, space="PSUM") as psTp, \
             tc.tile_pool(name="psC2", bufs=1, space="PSUM") as psC2p, \
             tc.tile_pool(name="psGC", bufs=1, space="PSUM") as psGC, \
             tc.tile_pool(name="psU", bufs=1, space="PSUM") as psUp, \
             tc.tile_pool(name="psC3", bufs=2, space="PSUM") as psC3:
            for g in range(NGRP):
                ps = pairs_of(g)
                gridB = psGC.tile([128, SUB], f32, tag="grid")
                if len(ps) == 1:
                    nc.vector.memset(gridB[0:4, :], 1.0)
                    nc.vector.memset(gridB[64:68, :], 1.0)
                h2fs = {}
                h1ps = {}
                for j, p in enumerate(ps):
                    # reconstruct channel-major h1 for this pair from the table
                    psH1 = psH1p.tile([64, 2, SUB], bf16, tag="psH1")
                    for s in range(2):
                        for jj in range(4):
                            b = 8 * p + 4 * s + jj
                            nc.tensor.transpose(
                                out=psH1[:, s, 128 * jj:128 * (jj + 1)],
                                in_=table[:, 128 * b:128 * b + 64],
                                identity=identb_sb[:],
                            )
                    h1pair = h1pp.tile([128, SUB], bf16, tag="h1p")
                    for s in range(2):
                        nc.scalar.copy(
                            out=h1pair[64 * s:64 * s + 64, :],
                            in_=psH1[:, s, :],
                        )
                    h1ps[j] = h1pair
                    # neighbor sum: transpose accumulator blocks to channel-major
                    psT = psTp.tile([64, 2, SUB], f32, tag="psT")
                    for s in range(2):
                        for jj in range(4):
                            b = 8 * p + 4 * s + jj
                            nc.tensor.transpose(
                                out=psT[:, s, 128 * jj:128 * (jj + 1)],
                                in_=acc[:, b & 1, b >> 1, :],
                                identity=identf_sb[:],
                            )
                    if dbg and p == 0:
                        pst_sb = h2fp.tile([64, 2, SUB], f32, tag="pstdbg")
                        nc.scalar.copy(out=pst_sb[:], in_=psT[:])
                        nc.sync.dma_start(out=pst_out[:], in_=pst_sb[:])
                    # center tap in its own PSUM bank
                    psC2 = psC2p.tile([128, SUB], f32, tag="psC2")
                    for s in range(2):
                        nc.tensor.matmul(
                            out=psC2[64 * s:64 * s + 64, :],
                            lhsT=w2c_sb[64 * s:64 * s + 64, :],
                            rhs=h1pair[64 * s:64 * s + 64, :],
                            start=True,
                            stop=True,
                            tile_position=(64 * s, 64 * s),
                        )
                    h2f = h2fp.tile([128, SUB], f32, tag="h2f")
                    for s in range(2):
                        nc.scalar.copy(
                            out=h2f[64 * s:64 * s + 64, :],
                            in_=psT[:, s, :],
                        )
                    nc.vector.scalar_tensor_tensor(
                        out=h2f[:], in0=psC2[:], scalar=ones_sb[:],
                        in1=h2f[:], op0=OP.mult, op1=OP.add,
                    )
                    h2fs[j] = h2f
                    if dbg:
                        nc.sync.dma_start(out=h2f_out[:, p, :], in_=h2f[:])
                        nc.sync.dma_start(out=h1p_out[:, p, :], in_=h1pair[:])
                    sq = sqp.tile([128, SUB], f32r, tag="sq")
                    nc.scalar.activation(out=sq[:], in_=h2f[:], func=AF.Square)
                    nc.tensor.matmul(
                        out=gridB[0:4, :],
                        lhsT=es_sb[:, j, :],
                        rhs=sq[:],
                        start=(j == 0),
                        stop=(j == len(ps) - 1),
                    )
                rstd2 = rsp.tile([4, SUB], f32, tag="rstd")
                nc.scalar.activation(
                    out=rstd2[:], in_=gridB[0:4, :], func=AF.Sqrt,
                    bias=eps_sb[0:4, :], scale=1.0 / C_MID,
                )
                nc.vector.reciprocal(out=rstd2[:], in_=rstd2[:])
                rsc2 = drp.tile([4, SUB], f32, tag="rsc")
                nc.sync.dma_start(out=rsc2[:], in_=rstd2[:])
                rb2 = rbp.tile([128, 2, SUB], f32, tag="rb1")
                for j in range(len(ps)):
                    for s in range(2):
                        nc.scalar.dma_start(
                            out=rb2[64 * s:64 * s + 64, j, :],
                            in_=_bcast_ap(
                                rsc2[2 * j + s:2 * j + s + 1, :],
                                [[0, 64], [1, SUB]],
                            ),
                        )
                h2ns = {}
                for j, p in enumerate(ps):
                    h2n = h2p.tile([128, SUB], bf16, tag="h2n")
                    h2ns[j] = h2n
                    nc.vector.scalar_tensor_tensor(
                        out=h2n[:], in0=h2fs[j][:], scalar=sc["g2"][:],
                        in1=rb2[:, j, :], op0=OP.mult, op1=OP.mult,
                    )
                    nc.scalar.activation(
                        out=h2n[:], in_=h2n[:], func=AF.Gelu, bias=sc["b2"][:],
                    )
                    u = psUp.tile([128, SUB], f32, tag="U")
                    for s in range(2):
                        nc.tensor.matmul(
                            out=u[64 * s:64 * s + 64, :],
                            lhsT=l_sb[64 * s:64 * s + 64, :],
                            rhs=h2n[64 * s:64 * s + 64, :],
                            start=True,
                            stop=True,
                            tile_position=(64 * s, 64 * s),
                        )
                    squ = sqp.tile([128, SUB], bf16, tag="squ")
                    nc.scalar.activation(out=squ[:], in_=u[:], func=AF.Square)
                    nc.tensor.matmul(
                        out=gridB[64:68, :],
                        lhsT=esb_sb[:, j, :],
                        rhs=squ[:],
                        start=(j == 0),
                        stop=(j == len(ps) - 1),
                        tile_position=(0, 64),
                    )
                rstd3 = rsp.tile([4, SUB], f32, tag="rstd3")
                nc.scalar.activation(
                    out=rstd3[:], in_=gridB[64:68, :], func=AF.Sqrt,
                    bias=eps_sb[0:4, :], scale=1.0,
                )
                nc.vector.reciprocal(out=rstd3[:], in_=rstd3[:])
                rsc3 = drp.tile([4, SUB], f32, tag="rsc")
                nc.sync.dma_start(out=rsc3[:], in_=rstd3[:])
                rb3 = rb3p.tile([128, 2, 2, SUB], f32, tag="rb3")
                for j in range(len(ps)):
                    for s in range(2):
                        nc.scalar.dma_start(
                            out=rb3[:, j, s, :],
                            in_=_bcast_ap(
                                rsc3[2 * j + s:2 * j + s + 1, :],
                                [[0, 128], [1, SUB]],
                            ),
                        )
                for j, p in enumerate(ps):
                    x2 = xp.tile([128, 2, PAIR], bf16, tag="xres")
                    nc.sync.dma_start(
                        out=x2[:],
                        in_=x_t[:, :, p * PAIR:(p + 1) * PAIR].rearrange(
                            "c p f -> p c f"
                        ),
                    )
                    y_sb = yp.tile([128, 2, PAIR], f32, tag="y")
                    for s in range(2):
                        for c in range(2):
                            a3 = psC3.tile([128, SUB], f32, tag="A3")
                            nc.tensor.matmul(
                                out=a3[:],
                                lhsT=w3_sb[64 * s:64 * s + 64,
                                           128 * c:128 * (c + 1)],
                                rhs=h2ns[j][64 * s:64 * s + 64, :],
                                start=True,
                                stop=True,
                                tile_position=(64 * s, 0),
                            )
                            nc.vector.scalar_tensor_tensor(
                                out=y_sb[:, c, SUB * s:SUB * (s + 1)],
                                in0=a3[:], scalar=g3_sb[:, c:c + 1],
                                in1=rb3[:, j, s, :], op0=OP.mult, op1=OP.mult,
                            )
                    yout = yop.tile([128, 2, PAIR], bf16, tag="yo")
                    for c in range(2):
                        nc.vector.scalar_tensor_tensor(
                            out=y_sb[:, c, :], in0=y_sb[:, c, :],
                            scalar=ones_sb[:],
                            in1=x2[:, c, :], op0=OP.mult, op1=OP.add,
                        )
                        nc.scalar.activation(
                            out=yout[:, c, :], in_=y_sb[:, c, :],
                            func=AF.Gelu, bias=b3_sb[:, c:c + 1],
                        )
                    nc.sync.dma_start(
                        out=y_t[:, :, p * PAIR:(p + 1) * PAIR].rearrange(
                            "c p f -> p c f"
                        ),
                        in_=yout[:],
                    )

    nc.compile()
    _NC_CACHE["nc"] = nc
    return nc


# ======================= host-side sharding =======================

def _components(nbr):
    """Connected-component labels via vectorized min-label propagation."""
    lab = np.arange(N, dtype=np.int64)
    ks = [k for k in range(9) if k != 4]
    valid = [(nbr[k] < N) for k in ks]
    nbrs = [nbr[k].astype(np.int64) for k in ks]
    for _ in range(200):
        new = lab.copy()
        for k in range(len(ks)):
            v = valid[k]
            cand = lab[nbrs[k][v]]
            np.minimum.at(new, np.nonzero(v)[0], cand)
        # also pull own label forward through one hop of indirection
        new = np.minimum(new, new[new])
        if np.array_equal(new, lab):
            break
        lab = new
    # canonicalize
    while True:
        new = lab[lab]
        if np.array_equal(new, lab):
            break
        lab = new
    return lab


def _shard(nbr):
    lab = _components(nbr)
    comp_ids, comp_inv, comp_sizes = np.unique(
        lab, return_inverse=True, return_counts=True
    )
    order = np.argsort(comp_sizes)[::-1]
    import heapq

    heap = [(0, c) for c in range(NCORES)]
    heapq.heapify(heap)
    comp_core = np.empty(len(comp_ids), dtype=np.int64)
    for ci in order:
        load, core = heapq.heappop(heap)
        comp_core[ci] = core
        heapq.heappush(heap, (load + int(comp_sizes[ci]), core))
    point_core = comp_core[comp_inv]
    ids_per_core = [np.nonzero(point_core == c)[0] for c in range(NCORES)]
    for c in range(NCORES):
        assert len(ids_per_core[c]) <= T, f"core {c} overloaded: {len(ids_per_core[c])}"
    return ids_per_core


def _wrap_idx(flat):
    """[n] int16 -> [128, n//16] (16-wrapped, replicated x8)."""
    n = flat.shape[0]
    w = flat.reshape(n // 16, 16).T
    return np.ascontiguousarray(np.tile(w, (8, 1)))


def _prep_core(x, nbr, ids):
    import ml_dtypes
    n = len(ids)
    glob2loc = np.full(N + 1, SENT, dtype=np.int64)
    glob2loc[ids] = np.arange(n)
    xl = np.zeros((T, C_IN), dtype=np.float32)
    xl[:n] = x[ids]
    x_t = np.ascontiguousarray(
        xl.T.reshape(2, 128, T).astype(ml_dtypes.bfloat16)
    )
    nbl = np.full((9, T), SENT, dtype=np.int64)
    nbl[:, :n] = glob2loc[np.where(nbr[:, ids] < N, nbr[:, ids], N)]
    gidx = np.empty((8, 128, NK // 16), dtype=np.int16)
    sidx = np.empty((8, 128, NK // 16), dtype=np.int16)
    for k8, k in enumerate(KS8):
        dst = np.nonzero(nbl[k, :n] != SENT)[0]
        srcl = nbl[k, dst]
        nk = len(dst)
        assert nk <= NK, f"tap {k}: {nk} edges > NK={NK}"
        gflat = np.full(NK, SENT, dtype=np.int16)
        gflat[:nk] = srcl.astype(np.int16)
        sflat = np.full(NK, TRASH, dtype=np.int16)
        sflat[:nk] = dst.astype(np.int16)
        gidx[k8] = _wrap_idx(gflat)
        sidx[k8] = _wrap_idx(sflat)
    return x_t, gidx, sidx, n


def _prep_weights(W1, W2, W3, g1, b1, g2, b2, g3, b3):
    import ml_dtypes
    W1 = np.asarray(W1, np.float64)
    W2 = np.asarray(W2, np.float64)
    W3 = np.asarray(W3, np.float64)

    def center(w, C):
        return w - w.mean(axis=-1, keepdims=True)

    W1c = center(W1, C_MID)          # [256, 64]
    W2c = center(W2, C_MID)          # [9, 64, 64]
    W3c = center(W3, C_IN)           # [64, 256]
    w1 = np.ascontiguousarray(
        W1c.reshape(2, 128, C_MID).transpose(1, 0, 2).astype(np.float32)
        .astype(ml_dtypes.bfloat16)
    )
    w2 = np.ascontiguousarray(
        W2c.transpose(1, 0, 2).astype(np.float32).astype(ml_dtypes.bfloat16)
    )  # [64, 9, 64]
    w2c = np.ascontiguousarray(
        np.tile(W2c[4].astype(np.float32), (2, 1)).astype(ml_dtypes.bfloat16)
    )  # [128, 64]
    w3 = np.ascontiguousarray(
        np.tile(W3c.astype(np.float32), (2, 1)).astype(ml_dtypes.bfloat16)
    )  # [128, 256] rows 64-127 replicated
    M3 = (W3c @ W3c.T) / C_IN
    L = np.linalg.cholesky(M3 + 1e-12 * np.eye(C_MID))
    lmat = np.ascontiguousarray(
        np.tile(L.astype(np.float32), (2, 1)).astype(ml_dtypes.bfloat16)
    )  # [128, 64]

    def rep2(v):
        return np.ascontiguousarray(
            np.tile(np.asarray(v, np.float32).reshape(C_MID), 2).reshape(128, 1)
        )

    g1r, b1r, g2r, b2r = rep2(g1), rep2(b1), rep2(g2), rep2(b2)
    g3r = np.ascontiguousarray(np.asarray(g3, np.float32).reshape(2, 128).T)
    b3r = np.ascontiguousarray(np.asarray(b3, np.float32).reshape(2, 128).T)
    es = np.zeros((128, 2, 4), np.float32)
    for j in range(2):
        for p in range(128):
            es[p, j, 2 * j + p // 64] = 1.0
    identf = np.eye(128, dtype=np.float32)
    identb = np.eye(128, dtype=np.float32).astype(ml_dtypes.bfloat16)
    return (w1, w2, w2c, w3, lmat, g1r, b1r, g2r, b2r, g3r, b3r, es,
            identf, identb)


def prep_in_maps(inputs):
    x = np.asarray(inputs["x"], np.float32)
    nbr = np.asarray(inputs["neighbor_idx"])
    (w1, w2, w2c, w3, lmat, g1r, b1r, g2r, b2r, g3r, b3r, es, identf,
     identb) = _prep_weights(
        inputs["W1"], inputs["W2"], inputs["W3"], inputs["g1"], inputs["b1"],
        inputs["g2"], inputs["b2"], inputs["g3"], inputs["b3"],
    )
    ids_per_core = _shard(nbr)
    in_maps = []
    metas = []
    for c in range(NCORES):
        x_t, gidx, sidx, n = _prep_core(x, nbr, ids_per_core[c])
        metas.append((ids_per_core[c], n))
        in_maps.append(
            dict(
                x_t=x_t, gidx=gidx, sidx=sidx, w1=w1, w2=w2, w2c=w2c, w3=w3,
                lmat=lmat, es=es, g1r=g1r, b1r=b1r, g2r=g2r, b2r=b2r,
                g3r=g3r, b3r=b3r, identf=identf, identb=identb,
            )
        )
    return in_maps, metas


def kernel(x, W1, W2, W3, g1, b1, g2, b2, g3, b3, neighbor_idx):
    in_maps, metas = prep_in_maps(
        dict(
            x=x, W1=W1, W2=W2, W3=W3, g1=g1, b1=b1, g2=g2, b2=b2,
            g3=g3, b3=b3, neighbor_idx=neighbor_idx,
        )
    )
    nc = build_nc()
    res = run_bass_kernel_spmd(nc, in_maps, core_ids=list(range(NCORES)))
    y = np.empty((N, C_IN), dtype=np.float32)
    for c in range(NCORES):
        yt = res.results[c]["y_t"]  # [2, 128, T] bf16
        ids, n = metas[c]
        yl = np.asarray(yt, dtype=np.float32).reshape(C_IN, T).T  # [T, 256]
        y[ids] = yl[:n]
    return y


# revision 14
# speedup vs baseline: 2.9038x; 1.3738x over previous
"""Trainium2 Bass kernel for nn_Bottleneck (sparse 3x3 gather-GEMM bottleneck block).

Strategy (8 NeuronCores, zero cross-core communication):
  - Connected components of the 19%-occupancy Moore-stencil graph are tiny;
    host assigns whole components to cores so every neighbor is core-local.
  - Channel-major compute; LN centering folded into conv weights; variance
    via ones-matmul; rstd broadcast via small DRAM-bounce DMAs; LN3 variance
    before conv3 via a Cholesky factor of W3c W3c^T / 256.
  - conv2 is EDGE-COMPACTED: only real edges per tap (~4.9k vs 25.6k) are
    gathered from a token-major table, multiplied in a flipped matmul
    (lhsT = gathered activations -> token-major products) and
    dma_scatter_add-ed (bf16) into SBUF parity accumulators. Center tap is a
    dense matmul over h1. Edges are split by destination half (lo/hi) so the
    epilogue for lo-tokens overlaps the hi-half scatter work (the Q7
    descriptor generator is the critical resource).
  - Table built with PE transposes (tensor engine) instead of DMA transposes.
  - x and y are bf16 in DRAM (host converts) to halve HBM traffic.
"""

import os
import sys

sys.path.insert(0, "/opt/trn_rl_repo")

import numpy as np

import concourse.bass as bass
import concourse.tile as tile
from concourse import bacc as bacc_mod
from concourse import library_config, mybir
from concourse.bass_utils import run_bass_kernel_spmd

# Problem constants (hardcoded per contract).
N = 200000
C_IN = 256
C_MID = 64
EPS = 1e-6
NCORES = 8

T = 25600
PAIR = 1024
SUB = 512
NPAIR = T // PAIR          # 25
NGRP = (NPAIR + 1) // 2    # 13 groups of <=2 pairs
NBLK = T // 128            # 200 table blocks
SENT = T                   # sentinel token id -> zeroed table rank
NRANK = NBLK + 1           # 201 ranks in the gather table
KS8 = [0, 1, 2, 3, 5, 6, 7, 8]  # non-center taps

SPLIT = 12288              # dst-half boundary (pair 12), 96 blocks
NKLO = 2560                # padded lo edges per tap (max measured 2420)
NKHI = 2688                # padded hi edges per tap (max measured 2529)
NCHLO = NKLO // 128        # 20
NCHHI = NKHI // 128        # 21
NGLO = SPLIT // 256 + 1    # 49 groups (48 real + trash)
NGHI = (T - SPLIT) // 256 + 1  # 53 groups (52 real + trash)
LO_TRASH = SPLIT           # -> group 48
HI_TRASH = T - SPLIT       # rebased -> group 52

f32 = mybir.dt.float32
f32r = mybir.dt.float32r
bf16 = mybir.dt.bfloat16
i16 = mybir.dt.int16
AF = mybir.ActivationFunctionType
OP = mybir.AluOpType

_NC_CACHE = {}


def _bcast_ap(src: bass.AP, ap_dims):
    """Manual access pattern with explicit [step, count] dims over src."""
    return bass.AP(tensor=src.tensor, offset=src.offset, ap=ap_dims)


def build_nc():
    if "nc" in _NC_CACHE:
        return _NC_CACHE["nc"]
    nc = bacc_mod.Bacc(None, target_bir_lowering=False, debug=False)

    x_t = nc.declare_dram_parameter("x_t", [2, 128, T], bf16, isOutput=False)
    w1 = nc.declare_dram_parameter("w1", [128, 2, C_MID], bf16, isOutput=False)
    w2 = nc.declare_dram_parameter("w2", [C_MID, 9, C_MID], bf16, isOutput=False)
    w2c = nc.declare_dram_parameter("w2c", [128, C_MID], bf16, isOutput=False)
    w3 = nc.declare_dram_parameter("w3", [128, C_IN], bf16, isOutput=False)
    lmat = nc.declare_dram_parameter("lmat", [128, C_MID], bf16, isOutput=False)
    es = nc.declare_dram_parameter("es", [128, 2, 4], f32r, isOutput=False)
    g1r = nc.declare_dram_parameter("g1r", [128, 1], f32, isOutput=False)
    b1r = nc.declare_dram_parameter("b1r", [128, 1], f32, isOutput=False)
    g2r = nc.declare_dram_parameter("g2r", [128, 1], f32, isOutput=False)
    b2r = nc.declare_dram_parameter("b2r", [128, 1], f32, isOutput=False)
    g3r = nc.declare_dram_parameter("g3r", [128, 2], f32, isOutput=False)
    b3r = nc.declare_dram_parameter("b3r", [128, 2], f32, isOutput=False)
    identb = nc.declare_dram_parameter("identb", [128, 128], bf16, isOutput=False)
    gidx = nc.declare_dram_parameter(
        "gidx", [8, 128, (NKLO + NKHI) // 16], i16, isOutput=False
    )
    sidx = nc.declare_dram_parameter(
        "sidx", [8, 128, (NKLO + NKHI) // 16], i16, isOutput=False
    )
    y_t = nc.declare_dram_parameter("y_t", [2, 128, T], bf16, isOutput=True)
    dbg = os.environ.get("KDBG", "0") == "1"
    if dbg:
        acclo_out = nc.declare_dram_parameter(
            "acclo_out", [128, 2, NGLO, C_MID], bf16, isOutput=True
        )
        acchi_out = nc.declare_dram_parameter(
            "acchi_out", [128, 2, NGHI, C_MID], bf16, isOutput=True
        )

    from contextlib import ExitStack

    with ExitStack() as ctx:
        tc = ctx.enter_context(tile.TileContext(nc))
        consts = ctx.enter_context(tc.tile_pool(name="consts", bufs=1))
        tablep = ctx.enter_context(tc.tile_pool(name="table", bufs=1))
        h1p = ctx.enter_context(tc.tile_pool(name="h1cm", bufs=1))
        accp = ctx.enter_context(tc.tile_pool(name="acc", bufs=1))
        xp = ctx.enter_context(tc.tile_pool(name="xin", bufs=2))
        sqp = ctx.enter_context(tc.tile_pool(name="sq", bufs=2))
        rsp = ctx.enter_context(tc.tile_pool(name="rstd", bufs=2))
        rbp = ctx.enter_context(tc.tile_pool(name="rbc", bufs=2))
        drp = ctx.enter_context(tc.tile_pool(name="drscratch", bufs=2, space="DRAM"))

        # ---- constants into SBUF ----
        w1_sb = consts.tile([128, 2, C_MID], bf16)
        nc.sync.dma_start(out=w1_sb[:], in_=w1[:])
        w2_sb = consts.tile([C_MID, 9, C_MID], bf16)
        nc.sync.dma_start(out=w2_sb[:], in_=w2[:])
        w2c_sb = consts.tile([128, C_MID], bf16)
        nc.sync.dma_start(out=w2c_sb[:], in_=w2c[:])
        w3_sb = consts.tile([128, C_IN], bf16)
        nc.sync.dma_start(out=w3_sb[:], in_=w3[:])
        l_sb = consts.tile([128, C_MID], bf16)
        nc.sync.dma_start(out=l_sb[:], in_=lmat[:])
        es_sb = consts.tile([128, 2, 4], f32r)
        nc.sync.dma_start(out=es_sb[:], in_=es[:])
        esb_sb = consts.tile([128, 2, 4], bf16)
        nc.gpsimd.dma_start(out=esb_sb[:], in_=es[:].bitcast(f32))
        identb_sb = consts.tile([128, 128], bf16)
        nc.sync.dma_start(out=identb_sb[:], in_=identb[:])
        sc = {}
        for name, src in (("g1", g1r), ("b1", b1r), ("g2", g2r), ("b2", b2r)):
            t_ = consts.tile([128, 1], f32, tag=f"sc_{name}")
            nc.sync.dma_start(out=t_[:], in_=src[:])
            sc[name] = t_
        g3_sb = consts.tile([128, 2], f32)
        nc.sync.dma_start(out=g3_sb[:], in_=g3r[:])
        b3_sb = consts.tile([128, 2], f32)
        nc.sync.dma_start(out=b3_sb[:], in_=b3r[:])
        eps_sb = consts.tile([128, 1], f32)
        nc.vector.memset(eps_sb[:], EPS)
        ones_sb = consts.tile([128, 1], f32)
        nc.vector.memset(ones_sb[:], 1.0)

        table = tablep.tile([128, NRANK * 128], bf16)   # point-major gather table
        nc.vector.memset(table[:], 0.0)
        h1cm = h1p.tile([128, NPAIR * SUB], bf16)       # channel-major h1n

        def pairs_of(g):
            return [p for p in (2 * g, 2 * g + 1) if p < NPAIR]

        # =================== PHASE A: conv1 + LN1 + GELU -> table ============
        with tc.tile_pool(name="psA", bufs=2, space="PSUM") as psA, \
             tc.tile_pool(name="psGA", bufs=2, space="PSUM") as psGA, \
             tc.tile_pool(name="psTab", bufs=4, space="PSUM") as psTabp:
            for g in range(NGRP):
                ps = pairs_of(g)
                gridA = psGA.tile([128, SUB], f32, tag="grid")
                if len(ps) == 1:
                    nc.vector.memset(gridA[0:4, :], 1.0)
                a1s = {}
                for j, p in enumerate(ps):
                    x_sb = xp.tile([128, 2, PAIR], bf16, tag="xin")
                    nc.sync.dma_start(
                        out=x_sb[:],
                        in_=x_t[:, :, p * PAIR:(p + 1) * PAIR].rearrange(
                            "c p f -> p c f"
                        ),
                    )
                    a1 = psA.tile([128, SUB], f32, tag="A1")
                    a1s[j] = a1
                    for s in range(2):
                        for c in range(2):
                            nc.tensor.matmul(
                                out=a1[64 * s:64 * s + 64, :],
                                lhsT=w1_sb[:, c, :],
                                rhs=x_sb[:, c, SUB * s:SUB * (s + 1)],
                                start=(c == 0),
                                stop=(c == 1),
                                tile_position=(0, 64 * s),
                            )
                    sq = sqp.tile([128, SUB], f32r, tag="sq")
                    nc.scalar.activation(out=sq[:], in_=a1[:], func=AF.Square)
                    nc.tensor.matmul(
                        out=gridA[0:4, :],
                        lhsT=es_sb[:, j, :],
                        rhs=sq[:],
                        start=(j == 0),
                        stop=(j == len(ps) - 1),
                    )
                rstd = rsp.tile([4, SUB], f32, tag="rstd")
                nc.scalar.activation(
                    out=rstd[:], in_=gridA[0:4, :], func=AF.Sqrt,
                    bias=eps_sb[0:4, :], scale=1.0 / C_MID,
                )
                nc.vector.reciprocal(out=rstd[:], in_=rstd[:])
                rsc = drp.tile([4, SUB], f32, tag="rsc")
                nc.sync.dma_start(out=rsc[:], in_=rstd[:])
                rb = rbp.tile([128, 2, SUB], f32, tag="rb1")
                for j in range(len(ps)):
                    for s in range(2):
                        nc.scalar.dma_start(
                            out=rb[64 * s:64 * s + 64, j, :],
                            in_=_bcast_ap(
                                rsc[2 * j + s:2 * j + s + 1, :],
                                [[0, 64], [1, SUB]],
                            ),
                        )
                for j, p in enumerate(ps):
                    h1s = h1cm[:, p * SUB:(p + 1) * SUB]
                    nc.vector.scalar_tensor_tensor(
                        out=h1s, in0=a1s[j][:], scalar=sc["g1"][:],
                        in1=rb[:, j, :], op0=OP.mult, op1=OP.mult,
                    )
                    nc.scalar.activation(
                        out=h1s, in_=h1s, func=AF.Gelu, bias=sc["b1"][:],
                    )
                    # token-major table blocks via PE transpose + DVE copy
                    for bl in range(8):
                        b = 8 * p + bl
                        s = bl // 4
                        c0 = p * SUB + 128 * (bl % 4)
                        ptab = psTabp.tile([128, C_MID], bf16, tag="ptab")
                        nc.tensor.transpose(
                            out=ptab[:],
                            in_=h1cm[64 * s:64 * s + 64, c0:c0 + 128],
                            identity=identb_sb[64 * s:64 * s + 64,
                                               64 * s:64 * s + 64],
                        )
                        nc.vector.tensor_copy(
                            out=table[:, 128 * b:128 * b + 64], in_=ptab[:]
                        )

        # ============ PHASE B/C: conv2 taps + epilogue (interleaved) =========
        acc_lo = accp.tile([128, 2, NGLO, C_MID], bf16)
        nc.vector.memset(acc_lo[:], 0.0)
        acc_hi = accp.tile([128, 2, NGHI, C_MID], bf16)
        nc.vector.memset(acc_hi[:], 0.0)
        nlo_reg = nc.gpsimd.to_reg(NKLO)
        nhi_reg = nc.gpsimd.to_reg(NKHI)
        zreg = nc.gpsimd.to_reg(0)

        with tc.tile_pool(name="gidxp", bufs=2) as gip, \
             tc.tile_pool(name="sidxp", bufs=2) as sip, \
             tc.tile_pool(name="gath", bufs=2) as gp, \
             tc.tile_pool(name="ssrc", bufs=2) as sp, \
             tc.tile_pool(name="h2fp", bufs=2) as h2fp, \
             tc.tile_pool(name="h2np", bufs=2) as h2p, \
             tc.tile_pool(name="rb3p", bufs=2) as rb3p, \
             tc.tile_pool(name="ysb", bufs=2) as yp, \
             tc.tile_pool(name="yout", bufs=2) as yop, \
             tc.tile_pool(name="psB", bufs=2, space="PSUM") as psB, \
             tc.tile_pool(name="psT", bufs=1, space="PSUM") as psTp, \
             tc.tile_pool(name="psC2", bufs=1, space="PSUM") as psC2p, \
             tc.tile_pool(name="psGC", bufs=1, space="PSUM") as psGC, \
             tc.tile_pool(name="psU", bufs=1, space="PSUM") as psUp, \
             tc.tile_pool(name="psC3", bufs=2, space="PSUM") as psC3:

            def emit_tap(k8, k, half):
                if half == 0:
                    nk, nch, off16, nreg, acc = NKLO, NCHLO, 0, nlo_reg, acc_lo
                else:
                    nk, nch, off16, nreg, acc = (
                        NKHI, NCHHI, NKLO // 16, nhi_reg, acc_hi
                    )
                gi = gip.tile([128, NKHI // 16], i16, tag="gi")
                nc.sync.dma_start(
                    out=gi[:, 0:nk // 16],
                    in_=gidx[k8, :, off16:off16 + nk // 16],
                )
                si = sip.tile([128, NKHI // 16], i16, tag="si")
                nc.sync.dma_start(
                    out=si[:, 0:nk // 16],
                    in_=sidx[k8, :, off16:off16 + nk // 16],
                )
                gath = gp.tile([128, 1, NKHI], bf16, tag="gath")
                nc.gpsimd.dma_gather(
                    out_ap=gath[:, :, 0:nk],
                    in_ap=table[:],
                    idxs_ap=gi[:, 0:nk // 16],
                    num_idxs=nk,
                    num_idxs_reg=nreg,
                    elem_size=128,
                    transpose=True,
                    sbuf_tokens_per_rank=128,
                    sbuf_free_dim_per_rank=256,
                    sbuf_free_dim_pad_per_rank=0,
                    sbuf_byte_offset=0,
                    single_packet=False,
                )
                src = sp.tile([128, NCHHI, C_MID], bf16, tag="src")
                for gch in range(0, nch, 8):
                    gg = min(8, nch - gch)
                    psb = psB.tile([128, SUB], f32, tag="psb")
                    for j in range(gg):
                        ch = gch + j
                        nc.tensor.matmul(
                            out=psb[:, 64 * j:64 * (j + 1)],
                            lhsT=gath[0:64, 0, 128 * ch:128 * (ch + 1)],
                            rhs=w2_sb[:, k, :],
                            start=True,
                            stop=True,
                        )
                    nc.scalar.copy(
                        out=src[:, gch:gch + gg, :].rearrange("p a b -> p (a b)"),
                        in_=psb[:, 0:64 * gg],
                    )
                nc.gpsimd.dma_scatter_add(
                    acc[:, 0, :, :],
                    src[:, 0:nch, :],
                    si[:, 0:nk // 16],
                    nk,
                    nreg,
                    C_MID,
                    parity_reg=zreg,
                    out_ap_other=acc[:, 1, :, :],
                    sbuf_tokens_per_rank=128,
                    single_packet=False,
                )

            def emit_epi_group(g):
                ps = pairs_of(g)
                gridB = psGC.tile([128, SUB], f32, tag="grid")
                if len(ps) == 1:
                    nc.vector.memset(gridB[0:4, :], 1.0)
                    nc.vector.memset(gridB[64:68, :], 1.0)
                h2fs = {}
                for j, p in enumerate(ps):
                    # neighbor sums: transpose accumulator blocks to ch-major
                    psT = psTp.tile([64, 2, SUB], bf16, tag="psT")
                    for s in range(2):
                        for jj in range(4):
                            b = 8 * p + 4 * s + jj
                            if b < 96:
                                blk = acc_lo[:, b & 1, b >> 1, :]
                            else:
                                b2 = b - 96
                                blk = acc_hi[:, b2 & 1, b2 >> 1, :]
                            nc.tensor.transpose(
                                out=psT[:, s, 128 * jj:128 * (jj + 1)],
                                in_=blk,
                                identity=identb_sb[:],
                            )
                    # center tap in its own PSUM bank
                    psC2 = psC2p.tile([128, SUB], f32, tag="psC2")
                    for s in range(2):
                        nc.tensor.matmul(
                            out=psC2[64 * s:64 * s + 64, :],
                            lhsT=w2c_sb[64 * s:64 * s + 64, :],
                            rhs=h1cm[64 * s:64 * s + 64,
                                     p * SUB:(p + 1) * SUB],
                            start=True,
                            stop=True,
                            tile_position=(64 * s, 64 * s),
                        )
                    h2f = h2fp.tile([128, SUB], f32, tag="h2f")
                    for s in range(2):
                        nc.scalar.copy(
                            out=h2f[64 * s:64 * s + 64, :],
                            in_=psT[:, s, :],
                        )
                    nc.vector.scalar_tensor_tensor(
                        out=h2f[:], in0=psC2[:], scalar=ones_sb[:],
                        in1=h2f[:], op0=OP.mult, op1=OP.add,
                    )
                    h2fs[j] = h2f
                    sq = sqp.tile([128, SUB], f32r, tag="sq")
                    nc.scalar.activation(out=sq[:], in_=h2f[:], func=AF.Square)
                    nc.tensor.matmul(
                        out=gridB[0:4, :],
                        lhsT=es_sb[:, j, :],
                        rhs=sq[:],
                        start=(j == 0),
                        stop=(j == len(ps) - 1),
                    )
                rstd2 = rsp.tile([4, SUB], f32, tag="rstd")
                nc.scalar.activation(
                    out=rstd2[:], in_=gridB[0:4, :], func=AF.Sqrt,
                    bias=eps_sb[0:4, :], scale=1.0 / C_MID,
                )
                nc.vector.reciprocal(out=rstd2[:], in_=rstd2[:])
                rsc2 = drp.tile([4, SUB], f32, tag="rsc")
                nc.sync.dma_start(out=rsc2[:], in_=rstd2[:])
                rb2 = rbp.tile([128, 2, SUB], f32, tag="rb1")
                for j in range(len(ps)):
                    for s in range(2):
                        nc.scalar.dma_start(
                            out=rb2[64 * s:64 * s + 64, j, :],
                            in_=_bcast_ap(
                                rsc2[2 * j + s:2 * j + s + 1, :],
                                [[0, 64], [1, SUB]],
                            ),
                        )
                h2ns = {}
                for j, p in enumerate(ps):
                    h2n = h2p.tile([128, SUB], bf16, tag="h2n")
                    h2ns[j] = h2n
                    nc.vector.scalar_tensor_tensor(
                        out=h2n[:], in0=h2fs[j][:], scalar=sc["g2"][:],
                        in1=rb2[:, j, :], op0=OP.mult, op1=OP.mult,
                    )
                    nc.scalar.activation(
                        out=h2n[:], in_=h2n[:], func=AF.Gelu, bias=sc["b2"][:],
                    )
                    u = psUp.tile([128, SUB], f32, tag="U")
                    for s in range(2):
                        nc.tensor.matmul(
                            out=u[64 * s:64 * s + 64, :],
                            lhsT=l_sb[64 * s:64 * s + 64, :],
                            rhs=h2n[64 * s:64 * s + 64, :],
                            start=True,
                            stop=True,
                            tile_position=(64 * s, 64 * s),
                        )
                    squ = sqp.tile([128, SUB], bf16, tag="squ")
                    nc.scalar.activation(out=squ[:], in_=u[:], func=AF.Square)
                    nc.tensor.matmul(
                        out=gridB[64:68, :],
                        lhsT=esb_sb[:, j, :],
                        rhs=squ[:],
                        start=(j == 0),
                        stop=(j == len(ps) - 1),
                        tile_position=(0, 64),
                    )
                rstd3 = rsp.tile([4, SUB], f32, tag="rstd3")
                nc.scalar.activation(
                    out=rstd3[:], in_=gridB[64:68, :], func=AF.Sqrt,
                    bias=eps_sb[0:4, :], scale=1.0,
                )
                nc.vector.reciprocal(out=rstd3[:], in_=rstd3[:])
                rsc3 = drp.tile([4, SUB], f32, tag="rsc")
                nc.sync.dma_start(out=rsc3[:], in_=rstd3[:])
                rb3 = rb3p.tile([128, 2, 2, SUB], f32, tag="rb3")
                for j in range(len(ps)):
                    for s in range(2):
                        nc.scalar.dma_start(
                            out=rb3[:, j, s, :],
                            in_=_bcast_ap(
                                rsc3[2 * j + s:2 * j + s + 1, :],
                                [[0, 128], [1, SUB]],
                            ),
                        )
                for j, p in enumerate(ps):
                    x2 = xp.tile([128, 2, PAIR], bf16, tag="xres")
                    nc.sync.dma_start(
                        out=x2[:],
                        in_=x_t[:, :, p * PAIR:(p + 1) * PAIR].rearrange(
                            "c p f -> p c f"
                        ),
                    )
                    y_sb = yp.tile([128, 2, PAIR], f32, tag="y")
                    for s in range(2):
                        for c in range(2):
                            a3 = psC3.tile([128, SUB], f32, tag="A3")
                            nc.tensor.matmul(
                                out=a3[:],
                                lhsT=w3_sb[64 * s:64 * s + 64,
                                           128 * c:128 * (c + 1)],
                                rhs=h2ns[j][64 * s:64 * s + 64, :],
                                start=True,
                                stop=True,
                                tile_position=(64 * s, 0),
                            )
                            nc.vector.scalar_tensor_tensor(
                                out=y_sb[:, c, SUB * s:SUB * (s + 1)],
                                in0=a3[:], scalar=g3_sb[:, c:c + 1],
                                in1=rb3[:, j, s, :], op0=OP.mult, op1=OP.mult,
                            )
                    yout = yop.tile([128, 2, PAIR], bf16, tag="yo")
                    for c in range(2):
                        nc.vector.scalar_tensor_tensor(
                            out=y_sb[:, c, :], in0=y_sb[:, c, :],
                            scalar=ones_sb[:],
                            in1=x2[:, c, :], op0=OP.mult, op1=OP.add,
                        )
                        nc.scalar.activation(
                            out=yout[:, c, :], in_=y_sb[:, c, :],
                            func=AF.Gelu, bias=b3_sb[:, c:c + 1],
                        )
                    nc.sync.dma_start(
                        out=y_t[:, :, p * PAIR:(p + 1) * PAIR].rearrange(
                            "c p f -> p c f"
                        ),
                        in_=yout[:],
                    )

            # lo-half taps first
            for k8, k in enumerate(KS8):
                emit_tap(k8, k, 0)
            if dbg:
                nc.sync.dma_start(out=acclo_out[:], in_=acc_lo[:])
            # hi-half taps interleaved with lo epilogue groups (0..5)
            for k8, k in enumerate(KS8):
                emit_tap(k8, k, 1)
                if k8 < 6:
                    emit_epi_group(k8)
            if dbg:
                nc.sync.dma_start(out=acchi_out[:], in_=acc_hi[:])
            # remaining epilogue groups (6..12 use acc_hi)
            for g in range(6, NGRP):
                emit_epi_group(g)

    nc.compile()
    _NC_CACHE["nc"] = nc
    return nc


# ======================= host-side sharding =======================

def _components(nbr):
    """Connected-component labels via vectorized min-label propagation."""
    lab = np.arange(N, dtype=np.int64)
    ks = [k for k in range(9) if k != 4]
    valid = [(nbr[k] < N) for k in ks]
    nbrs = [nbr[k].astype(np.int64) for k in ks]
    for _ in range(200):
        new = lab.copy()
        for k in range(len(ks)):
            v = valid[k]
            cand = lab[nbrs[k][v]]
            np.minimum.at(new, np.nonzero(v)[0], cand)
        new = np.minimum(new, new[new])
        if np.array_equal(new, lab):
            break
        lab = new
    while True:
        new = lab[lab]
        if np.array_equal(new, lab):
            break
        lab = new
    return lab


def _shard(nbr):
    lab = _components(nbr)
    comp_ids, comp_inv, comp_sizes = np.unique(
        lab, return_inverse=True, return_counts=True
    )
    order = np.argsort(comp_sizes)[::-1]
    import heapq

    heap = [(0, c) for c in range(NCORES)]
    heapq.heapify(heap)
    comp_core = np.empty(len(comp_ids), dtype=np.int64)
    for ci in order:
        load, core = heapq.heappop(heap)
        comp_core[ci] = core
        heapq.heappush(heap, (load + int(comp_sizes[ci]), core))
    point_core = comp_core[comp_inv]
    ids_per_core = [np.nonzero(point_core == c)[0] for c in range(NCORES)]
    for c in range(NCORES):
        assert len(ids_per_core[c]) <= T, f"core {c} overloaded: {len(ids_per_core[c])}"
    return ids_per_core


def _wrap_idx(flat):
    """[n] int16 -> [128, n//16] (16-wrapped, replicated x8)."""
    n = flat.shape[0]
    w = flat.reshape(n // 16, 16).T
    return np.ascontiguousarray(np.tile(w, (8, 1)))


def _prep_core(x, nbr, ids):
    import ml_dtypes
    n = len(ids)
    glob2loc = np.full(N + 1, SENT, dtype=np.int64)
    glob2loc[ids] = np.arange(n)
    xl = np.zeros((T, C_IN), dtype=np.float32)
    xl[:n] = x[ids]
    x_t = np.ascontiguousarray(
        xl.T.reshape(2, 128, T).astype(ml_dtypes.bfloat16)
    )
    nbl = np.full((9, T), SENT, dtype=np.int64)
    nbl[:, :n] = glob2loc[np.where(nbr[:, ids] < N, nbr[:, ids], N)]
    gidx = np.empty((8, 128, (NKLO + NKHI) // 16), dtype=np.int16)
    sidx = np.empty((8, 128, (NKLO + NKHI) // 16), dtype=np.int16)
    for k8, k in enumerate(KS8):
        dst = np.nonzero(nbl[k, :n] != SENT)[0]
        srcl = nbl[k, dst]
        lo = dst < SPLIT
        dlo, slo = dst[lo], srcl[lo]
        dhi, shi = dst[~lo] - SPLIT, srcl[~lo]
        assert len(dlo) <= NKLO, f"tap {k}: {len(dlo)} lo edges > {NKLO}"
        assert len(dhi) <= NKHI, f"tap {k}: {len(dhi)} hi edges > {NKHI}"
        gf = np.full(NKLO + NKHI, SENT, dtype=np.int16)
        gf[:len(slo)] = slo.astype(np.int16)
        gf[NKLO:NKLO + len(shi)] = shi.astype(np.int16)
        sf = np.empty(NKLO + NKHI, dtype=np.int16)
        sf[:NKLO] = LO_TRASH
        sf[:len(dlo)] = dlo.astype(np.int16)
        sf[NKLO:] = HI_TRASH
        sf[NKLO:NKLO + len(dhi)] = dhi.astype(np.int16)
        gidx[k8] = np.concatenate(
            [_wrap_idx(gf[:NKLO]), _wrap_idx(gf[NKLO:])], axis=1
        )
        sidx[k8] = np.concatenate(
            [_wrap_idx(sf[:NKLO]), _wrap_idx(sf[NKLO:])], axis=1
        )
    return x_t, gidx, sidx, n


def _prep_weights(W1, W2, W3, g1, b1, g2, b2, g3, b3):
    import ml_dtypes
    W1 = np.asarray(W1, np.float64)
    W2 = np.asarray(W2, np.float64)
    W3 = np.asarray(W3, np.float64)

    def center(w, C):
        return w - w.mean(axis=-1, keepdims=True)

    W1c = center(W1, C_MID)          # [256, 64]
    W2cc = center(W2, C_MID)         # [9, 64, 64]
    W3c = center(W3, C_IN)           # [64, 256]
    w1 = np.ascontiguousarray(
        W1c.reshape(2, 128, C_MID).transpose(1, 0, 2).astype(np.float32)
        .astype(ml_dtypes.bfloat16)
    )
    w2 = np.ascontiguousarray(
        W2cc.transpose(1, 0, 2).astype(np.float32).astype(ml_dtypes.bfloat16)
    )  # [64, 9, 64]
    w2c = np.ascontiguousarray(
        np.tile(W2cc[4].astype(np.float32), (2, 1)).astype(ml_dtypes.bfloat16)
    )  # [128, 64]
    w3 = np.ascontiguousarray(
        np.tile(W3c.astype(np.float32), (2, 1)).astype(ml_dtypes.bfloat16)
    )  # [128, 256] rows 64-127 replicated
    M3 = (W3c @ W3c.T) / C_IN
    L = np.linalg.cholesky(M3 + 1e-12 * np.eye(C_MID))
    lmat = np.ascontiguousarray(
        np.tile(L.astype(np.float32), (2, 1)).astype(ml_dtypes.bfloat16)
    )  # [128, 64]

    def rep2(v):
        return np.ascontiguousarray(
            np.tile(np.asarray(v, np.float32).reshape(C_MID), 2).reshape(128, 1)
        )

    g1r, b1r, g2r, b2r = rep2(g1), rep2(b1), rep2(g2), rep2(b2)
    g3r = np.ascontiguousarray(np.asarray(g3, np.float32).reshape(2, 128).T)
    b3r = np.ascontiguousarray(np.asarray(b3, np.float32).reshape(2, 128).T)
    es = np.zeros((128, 2, 4), np.float32)
    for j in range(2):
        for p in range(128):
            es[p, j, 2 * j + p // 64] = 1.0
    identb = np.eye(128, dtype=np.float32).astype(ml_dtypes.bfloat16)
    return (w1, w2, w2c, w3, lmat, g1r, b1r, g2r, b2r, g3r, b3r, es, identb)


def prep_in_maps(inputs):
    x = np.asarray(inputs["x"], np.float32)
    nbr = np.asarray(inputs["neighbor_idx"])
    (w1, w2, w2c, w3, lmat, g1r, b1r, g2r, b2r, g3r, b3r, es,
     identb) = _prep_weights(
        inputs["W1"], inputs["W2"], inputs["W3"], inputs["g1"], inputs["b1"],
        inputs["g2"], inputs["b2"], inputs["g3"], inputs["b3"],
    )
    ids_per_core = _shard(nbr)
    in_maps = []
    metas = []
    for c in range(NCORES):
        x_t, gidx, sidx, n = _prep_core(x, nbr, ids_per_core[c])
        metas.append((ids_per_core[c], n))
        in_maps.append(
            dict(
                x_t=x_t, gidx=gidx, sidx=sidx, w1=w1, w2=w2, w2c=w2c, w3=w3,
                lmat=lmat, es=es, g1r=g1r, b1r=b1r, g2r=g2r, b2r=b2r,
                g3r=g3r, b3r=b3r, identb=identb,
            )
        )
    return in_maps, metas


def kernel(x, W1, W2, W3, g1, b1, g2, b2, g3, b3, neighbor_idx):
    in_maps, metas = prep_in_maps(
        dict(
            x=x, W1=W1, W2=W2, W3=W3, g1=g1, b1=b1, g2=g2, b2=b2,
            g3=g3, b3=b3, neighbor_idx=neighbor_idx,
        )
    )
    nc = build_nc()
    res = run_bass_kernel_spmd(nc, in_maps, core_ids=list(range(NCORES)))
    y = np.empty((N, C_IN), dtype=np.float32)
    for c in range(NCORES):
        yt = res.results[c]["y_t"]  # [2, 128, T] bf16
        ids, n = metas[c]
        yl = np.asarray(yt, dtype=np.float32).reshape(C_IN, T).T  # [T, 256]
        y[ids] = yl[:n]
    return y
